# revision 46
# baseline (speedup 1.0000x reference)
"""Trainium2 Bass kernel for windowed embedding lookup (nn_AttentionLayer).

Computation:
  out[b,s,e] = sum_k w[k,e] * data[snip_b, clip(inputs[b,s]+k-5, 0, 165), 0, e]

Strategy (data-parallel over batch, 2 batches per core on 8 cores):
  1. Per batch, load the snippet's table slice T [166,768] (transposed
     [e,p] layout staged by the host) via a dynamic-offset DMA
     (snippet id read into a register with values_load).
  2. Compute the 11-tap clip-padded convolution
     C[p,e] = sum_k w[k,e]*T[clip(p+k-5),e] on the TensorEngine as
     PSUM-accumulated matmuls: lhsT = shifted T-window (stationary),
     rhs = diag(w[k, e-chunk]) (host-staged diagonal matrices), which
     emits C directly in [p,e] layout.
  3. Gather rows out[s] = C[inputs[s]] as a one-hot matmul on TensorE
     (one-hot built with iota + is_equal against the replicated input row).
  4. Drain PSUM on DVE/ACT and DMA the [1126,768] f32 result to DRAM.

The host only does layout transforms (slice/transpose/reshape/placing
weight values on diagonals) and sharding; all arithmetic runs on device.
Measured: ~61-64 us HW exec for the full 8-core SPMD NEFF (vs ~15 us
fixed Tile/runtime floor), rel err ~3e-3 (bf16 table/one-hot quantization).
"""

import sys

for _p in ("/opt/trn_rl_repo",):
    if _p not in sys.path:
        sys.path.insert(0, _p)

import numpy as np

N_CORES = 8
B = 16
BPC = B // N_CORES  # batches per core
S = 1126
E = 768
EC = 6  # number of 128-wide e chunks
P = 166  # table positions
PPAD = 176  # padded positions (5 on each side)
W = 11
NSNIP = 100
MTILES = (S + 127) // 128  # 9

_cache = {}


def _build(debug=False):
    import concourse.bass as bass
    import concourse.mybir as mybir
    import concourse.tile as tile
    from concourse import bacc
    from concourse.masks import make_identity

    f32 = mybir.dt.float32
    bf16 = mybir.dt.bfloat16
    i32 = mybir.dt.int32
    AOT = mybir.AluOpType

    nc = bacc.Bacc()
    dbg = {}
    if debug:
        dbg["t2"] = nc.declare_dram_parameter(
            "dbg_t2", [128, EC * PPAD], f32, isOutput=True
        )
        dbg["rows"] = nc.declare_dram_parameter(
            "dbg_rows", [128, 1], i32, isOutput=True
        )
        dbg["inpb"] = nc.declare_dram_parameter(
            "dbg_inpb", [128, S], f32, isOutput=True
        )
        dbg["oh0"] = nc.declare_dram_parameter(
            "dbg_oh0", [128, S], f32, isOutput=True
        )
        dbg["c2"] = nc.declare_dram_parameter(
            "dbg_c2", [128, EC * P], f32, isOutput=True
        )
        dbg["ccat0"] = nc.declare_dram_parameter(
            "dbg_ccat0", [128, E], f32, isOutput=True
        )

    meta = nc.declare_dram_parameter(
        "meta", [1, BPC + BPC * S], i32, isOutput=False
    )
    # row (snip*128 + i) holds [c*166 + p] -> data[snip, p, 0, c*128 + i]
    dataT2 = nc.declare_dram_parameter(
        "dataT2", [NSNIP * 128, EC * P], f32, isOutput=False
    )
    # diagonal weight matrices: [i, (c*11+k)*128 + j] = w[k, c*128+i] iff i==j
    bf16_dt = mybir.dt.bfloat16
    diagw = nc.declare_dram_parameter(
        "diagw", [128, EC * W * 128], bf16_dt, isOutput=False
    )
    out = nc.declare_dram_parameter("out", [BPC * S, E], f32, isOutput=True)

    with tile.TileContext(nc) as tc:
        with (
            tc.tile_pool(name="const", bufs=1) as constp,
            tc.tile_pool(name="work", bufs=2) as workp,
            tc.tile_pool(name="mm", bufs=2) as mmp,
            tc.tile_pool(name="ob", bufs=4) as obp,
            tc.tile_pool(name="psum_c", bufs=2, space="PSUM") as psumc,
            tc.tile_pool(name="psum_mm", bufs=3, space="PSUM") as psummm,
        ):
            ones1 = constp.tile([1, 128], bf16)
            nc.vector.memset(ones1[:], 1.0)

            iota_i = constp.tile([128, 1], i32)
            nc.gpsimd.iota(iota_i[:], [[1, 1]], base=0, channel_multiplier=1)
            iota_f = constp.tile([128, 1], f32)
            nc.vector.tensor_copy(iota_f[:], iota_i[:])
            iota_f_hi = constp.tile([128, 1], f32)
            nc.vector.tensor_scalar_add(iota_f_hi[:], iota_f[:], 128.0)

            # warm up the SWDGE dynamic-DMA path while waiting for meta
            warm = constp.tile([1, 4], f32)
            nc.gpsimd.dma_start(out=warm[:], in_=dataT2[0:1, 0:4])
            metat = constp.tile([1, BPC + BPC * S], i32)
            nc.sync.dma_start(out=metat[:], in_=meta[:])

            diagb = constp.tile([128, EC * W, 128], bf16)

            def diag_chunk(c):
                nc.sync.dma_start(
                    out=diagb[:, c * W : (c + 1) * W, :],
                    in_=diagw[:, c * W * 128 : (c + 1) * W * 128].rearrange(
                        "p (k j) -> p k j", j=128
                    ),
                )

            def gather_t2(b):
                snip_val = nc.values_load(
                    metat[0:1, b : b + 1],
                    min_val=0,
                    max_val=NSNIP - 1,
                    skip_runtime_bounds_check=True,
                )
                t2raw = workp.tile([128, EC * P], f32, tag="t2raw")
                nc.gpsimd.dma_start(
                    out=t2raw[:, :],
                    in_=dataT2[bass.ts(snip_val, 128), :],
                )
                t2 = workp.tile([128, EC, PPAD], bf16, tag="t2")
                nc.vector.tensor_copy(
                    t2[:, :, 5 : 5 + P],
                    t2raw[:, :].rearrange("p (c q) -> p c q", q=P),
                )
                for c in range(EC):
                    nc.vector.tensor_copy(
                        t2[:, c, 0:5], t2[:, c, 5:6].to_broadcast([128, 5])
                    )
                    nc.vector.tensor_copy(
                        t2[:, c, 5 + P : PPAD],
                        t2[:, c, 4 + P : 5 + P].to_broadcast([128, 5]),
                    )
                return t2

            def inpr_cast(b):
                inpr_f = workp.tile([1, S], bf16, tag=f"inprf{b}")
                nc.vector.tensor_copy(
                    inpr_f[:], metat[0:1, BPC + b * S : BPC + (b + 1) * S]
                )
                return inpr_f

            def onehot(b, inpr_f):
                inpb_f = workp.tile([128, S], bf16, tag="inpbf")
                for n0 in range(0, S, 512):
                    nw = min(512, S - n0)
                    ps_in = psumc.tile([128, 512], f32, tag="pc")
                    nc.tensor.matmul(
                        out=ps_in[:, :nw],
                        lhsT=ones1[:, :],
                        rhs=inpr_f[:, n0 : n0 + nw],
                        start=True,
                        stop=True,
                    )
                    nc.vector.tensor_copy(
                        inpb_f[:, n0 : n0 + nw], ps_in[:, :nw]
                    )
                oh0 = mmp.tile([128, S], bf16, tag="oh0")
                oh1 = mmp.tile([128, S], bf16, tag="oh1")
                nc.vector.tensor_scalar(
                    oh0[:], inpb_f[:], iota_f[:, :1], None, AOT.is_equal
                )
                nc.vector.tensor_scalar(
                    oh1[:], inpb_f[:], iota_f_hi[:, :1], None, AOT.is_equal
                )
                return oh0, oh1

            # ---- setup: input casts, table gathers, weights, one-hots
            inpr_b = [inpr_cast(0), inpr_cast(1)]
            t2_b = [gather_t2(0), gather_t2(1)]
            for c in range(EC):
                diag_chunk(c)
            oh_b = [onehot(0, inpr_b[0]), onehot(1, inpr_b[1])]

            ccat_b = []
            for b in range(BPC):
                t2 = t2_b[b]

                # ---- 11-tap conv on TensorE, output directly in [p, e]:
                # out[p', e'] = sum_i t2[i, c, off+p'+k] * diag_ck[i, e']
                ccat0 = mmp.tile([128, E], bf16, tag=f"c0_{b}")
                ccat1 = mmp.tile([128, E], bf16, tag=f"c1_{b}")
                nc.vector.memzero(ccat1[:])
                # groups: (pc, c-range, drain engine)
                groups = (
                    (0, range(0, 3), "v"),
                    (0, range(3, EC), "v"),
                    (1, range(0, 3), "v"),
                    (1, range(3, EC), "v"),
                )
                for pc, crange, eng in groups:
                    pcw = 128 if pc == 0 else P - 128
                    gw = len(crange) * 128
                    psc = psumc.tile([128, 512], f32, tag="pc")
                    for ci, c in enumerate(crange):
                        for k in range(W):
                            nc.tensor.matmul(
                                out=psc[:pcw, ci * 128 : (ci + 1) * 128],
                                lhsT=t2[:, c, k + pc * 128 : k + pc * 128 + pcw],
                                rhs=diagb[:, c * W + k, :],
                                start=(k == 0),
                                stop=(k == W - 1),
                            )
                    cdst = ccat0 if pc == 0 else ccat1
                    c0 = crange.start * 128
                    if eng == "v":
                        nc.vector.tensor_copy(
                            cdst[:pcw, c0 : c0 + gw], psc[:pcw, :gw]
                        )
                    else:
                        nc.scalar.copy(
                            cdst[:pcw, c0 : c0 + gw], psc[:pcw, :gw]
                        )
                ccat_b.append((ccat0, ccat1))
                if debug and b == 0:
                    nc.gpsimd.dma_start(out=dbg["ccat0"][:], in_=ccat0[:])

                oh0, oh1 = oh_b[b]
                # ---- gather matmul: out[s, e] = sum_p oh[p, s] * C[p, e]
                for m in range(MTILES):
                    mw = min(128, S - m * 128)
                    pso = psummm.tile([128, E], f32, tag="po")
                    for oh, cc, st in ((oh0, ccat0, True), (oh1, ccat1, False)):
                        for n0, nw in ((0, 512), (512, 256)):
                            nc.tensor.matmul(
                                out=pso[:mw, n0 : n0 + nw],
                                lhsT=oh[:, m * 128 : m * 128 + mw],
                                rhs=cc[:, n0 : n0 + nw],
                                start=st,
                                stop=not st,
                            )
                    ob = obp.tile([128, E], f32, tag="ob")
                    if m % 3 == 0:
                        nc.vector.tensor_copy(ob[:mw, :], pso[:mw, :])
                    else:
                        nc.scalar.copy(ob[:mw, :], pso[:mw, :])
                    nc.sync.dma_start(
                        out=out[b * S + m * 128 : b * S + m * 128 + mw, :],
                        in_=ob[:mw, :],
                    )

    nc.finalize()
    return nc


def _get_nc():
    if "nc" not in _cache:
        _cache["nc"] = _build()
    return _cache["nc"]


def _prep_shared(data, w):
    # layout-only host staging (no arithmetic)
    d0 = np.asarray(data, dtype=np.float32)[:, :, 0, :]  # [100, 166, 768]
    dT = np.transpose(d0, (0, 2, 1))  # [100, 768, 166]
    dT = (
        dT.reshape(NSNIP, EC, 128, P)
        .transpose(0, 2, 1, 3)
        .reshape(NSNIP * 128, EC * P)
    )
    dataT2 = np.ascontiguousarray(dT, dtype=np.float32)
    wT = np.asarray(w, dtype=np.float32).T  # [768, 11]
    w2 = wT.reshape(EC, 128, W).transpose(1, 0, 2)  # [128, EC, W]
    import ml_dtypes

    diagw = np.zeros((128, EC * W, 128), dtype=ml_dtypes.bfloat16)
    ii = np.arange(128)
    diagw[ii, :, ii] = w2.reshape(128, EC * W).astype(ml_dtypes.bfloat16)
    diagw = np.ascontiguousarray(diagw.reshape(128, EC * W * 128))
    return dataT2, diagw


def kernel(inputs, code_snippet_id, data, w, _trace=False):
    from concourse.bass_utils import run_bass_kernel_spmd

    nc = _get_nc()
    inputs = np.asarray(inputs, dtype=np.int32)
    code_snippet_id = np.asarray(code_snippet_id, dtype=np.int32)
    dataT2, diagw = _prep_shared(data, w)

    in_maps = []
    for ci in range(N_CORES):
        b0 = ci * BPC
        in_maps.append(
            {
                "meta": np.ascontiguousarray(
                    np.concatenate(
                        [
                            code_snippet_id[b0 : b0 + BPC].reshape(-1),
                            inputs[b0 : b0 + BPC].reshape(-1),
                        ]
                    ).reshape(1, -1)
                ),
                "dataT2": dataT2,
                "diagw": diagw,
            }
        )

    res = run_bass_kernel_spmd(
        nc, in_maps, core_ids=list(range(N_CORES)), trace=_trace
    )
    _cache["last_results"] = res
    out = np.concatenate(
        [res.results[i]["out"].reshape(BPC, S, E) for i in range(N_CORES)],
        axis=0,
    ).astype(np.float32)
    return out


# revision 47
# speedup vs baseline: 1.0012x; 1.0012x over previous
"""Trainium2 Bass kernel for windowed embedding lookup (nn_AttentionLayer).

Computation:
  out[b,s,e] = sum_k w[k,e] * data[snip_b, clip(inputs[b,s]+k-5, 0, 165), 0, e]

Strategy (data-parallel over batch, 2 batches per core on 8 cores):
  1. Per batch, load the snippet's table slice T [166,768] (transposed
     [e,p] layout staged by the host) via a dynamic-offset DMA
     (snippet id read into a register with values_load).
  2. Compute the 11-tap clip-padded convolution
     C[p,e] = sum_k w[k,e]*T[clip(p+k-5),e] on the TensorEngine as
     PSUM-accumulated matmuls: lhsT = shifted T-window (stationary),
     rhs = diag(w[k, e-chunk]) (host-staged diagonal matrices), which
     emits C directly in [p,e] layout.
  3. Gather rows out[s] = C[inputs[s]] as a one-hot matmul on TensorE
     (one-hot built with iota + is_equal against the replicated input row).
  4. Drain PSUM on DVE/ACT and DMA the [1126,768] f32 result to DRAM.

The host only does layout transforms (slice/transpose/reshape/placing
weight values on diagonals) and sharding; all arithmetic runs on device.
Measured: ~61-64 us HW exec for the full 8-core SPMD NEFF (vs ~15 us
fixed Tile/runtime floor), rel err ~3e-3 (bf16 table/one-hot quantization).
"""

import sys

for _p in ("/opt/trn_rl_repo",):
    if _p not in sys.path:
        sys.path.insert(0, _p)

import numpy as np

N_CORES = 8
B = 16
BPC = B // N_CORES  # batches per core
S = 1126
E = 768
EC = 6  # number of 128-wide e chunks
P = 166  # table positions
PPAD = 176  # padded positions (5 on each side)
W = 11
NSNIP = 100
MTILES = (S + 127) // 128  # 9

_cache = {}


def _build(debug=False):
    import concourse.bass as bass
    import concourse.mybir as mybir
    import concourse.tile as tile
    from concourse import bacc
    from concourse.masks import make_identity

    f32 = mybir.dt.float32
    bf16 = mybir.dt.bfloat16
    i32 = mybir.dt.int32
    AOT = mybir.AluOpType

    nc = bacc.Bacc()
    dbg = {}
    if debug:
        dbg["t2"] = nc.declare_dram_parameter(
            "dbg_t2", [128, EC * PPAD], f32, isOutput=True
        )
        dbg["rows"] = nc.declare_dram_parameter(
            "dbg_rows", [128, 1], i32, isOutput=True
        )
        dbg["inpb"] = nc.declare_dram_parameter(
            "dbg_inpb", [128, S], f32, isOutput=True
        )
        dbg["oh0"] = nc.declare_dram_parameter(
            "dbg_oh0", [128, S], f32, isOutput=True
        )
        dbg["c2"] = nc.declare_dram_parameter(
            "dbg_c2", [128, EC * P], f32, isOutput=True
        )
        dbg["ccat0"] = nc.declare_dram_parameter(
            "dbg_ccat0", [128, E], f32, isOutput=True
        )

    meta = nc.declare_dram_parameter(
        "meta", [1, BPC + BPC * S], i32, isOutput=False
    )
    # row (snip*128 + i) holds [c*166 + p] -> data[snip, p, 0, c*128 + i]
    dataT2 = nc.declare_dram_parameter(
        "dataT2", [NSNIP * 128, EC * P], f32, isOutput=False
    )
    # diagonal weight matrices: [i, (c*11+k)*128 + j] = w[k, c*128+i] iff i==j
    bf16_dt = mybir.dt.bfloat16
    diagw = nc.declare_dram_parameter(
        "diagw", [128, EC * W * 128], bf16_dt, isOutput=False
    )
    out = nc.declare_dram_parameter("out", [BPC * S, E], f32, isOutput=True)

    with tile.TileContext(nc) as tc:
        with (
            tc.tile_pool(name="const", bufs=1) as constp,
            tc.tile_pool(name="work", bufs=2) as workp,
            tc.tile_pool(name="mm", bufs=2) as mmp,
            tc.tile_pool(name="ob", bufs=6) as obp,
            tc.tile_pool(name="psum_c", bufs=2, space="PSUM") as psumc,
            tc.tile_pool(name="psum_mm", bufs=3, space="PSUM") as psummm,
        ):
            ones1 = constp.tile([1, 128], bf16)
            nc.vector.memset(ones1[:], 1.0)

            iota_i = constp.tile([128, 1], i32)
            nc.gpsimd.iota(iota_i[:], [[1, 1]], base=0, channel_multiplier=1)
            iota_f = constp.tile([128, 1], f32)
            nc.vector.tensor_copy(iota_f[:], iota_i[:])
            iota_f_hi = constp.tile([128, 1], f32)
            nc.vector.tensor_scalar_add(iota_f_hi[:], iota_f[:], 128.0)

            # warm up the SWDGE dynamic-DMA path while waiting for meta
            warm = constp.tile([1, 4], f32)
            nc.gpsimd.dma_start(out=warm[:], in_=dataT2[0:1, 0:4])
            metat = constp.tile([1, BPC + BPC * S], i32)
            nc.sync.dma_start(out=metat[:], in_=meta[:])

            diagb = constp.tile([128, EC * W, 128], bf16)

            def diag_chunk(c):
                nc.sync.dma_start(
                    out=diagb[:, c * W : (c + 1) * W, :],
                    in_=diagw[:, c * W * 128 : (c + 1) * W * 128].rearrange(
                        "p (k j) -> p k j", j=128
                    ),
                )

            def gather_t2(b):
                snip_val = nc.values_load(
                    metat[0:1, b : b + 1],
                    min_val=0,
                    max_val=NSNIP - 1,
                    skip_runtime_bounds_check=True,
                )
                t2raw = workp.tile([128, EC * P], f32, tag="t2raw")
                nc.gpsimd.dma_start(
                    out=t2raw[:, :],
                    in_=dataT2[bass.ts(snip_val, 128), :],
                )
                t2 = workp.tile([128, EC, PPAD], bf16, tag="t2")
                nc.vector.tensor_copy(
                    t2[:, :, 5 : 5 + P],
                    t2raw[:, :].rearrange("p (c q) -> p c q", q=P),
                )
                for c in range(EC):
                    nc.vector.tensor_copy(
                        t2[:, c, 0:5], t2[:, c, 5:6].to_broadcast([128, 5])
                    )
                    nc.vector.tensor_copy(
                        t2[:, c, 5 + P : PPAD],
                        t2[:, c, 4 + P : 5 + P].to_broadcast([128, 5]),
                    )
                return t2

            def inpr_cast(b):
                inpr_f = workp.tile([1, S], bf16, tag=f"inprf{b}")
                nc.vector.tensor_copy(
                    inpr_f[:], metat[0:1, BPC + b * S : BPC + (b + 1) * S]
                )
                return inpr_f

            def onehot(b, inpr_f):
                inpb_f = workp.tile([128, S], bf16, tag="inpbf")
                for n0 in range(0, S, 512):
                    nw = min(512, S - n0)
                    ps_in = psumc.tile([128, 512], f32, tag="pc")
                    nc.tensor.matmul(
                        out=ps_in[:, :nw],
                        lhsT=ones1[:, :],
                        rhs=inpr_f[:, n0 : n0 + nw],
                        start=True,
                        stop=True,
                    )
                    nc.vector.tensor_copy(
                        inpb_f[:, n0 : n0 + nw], ps_in[:, :nw]
                    )
                oh0 = mmp.tile([128, S], bf16, tag="oh0")
                oh1 = mmp.tile([128, S], bf16, tag="oh1")
                nc.vector.tensor_scalar(
                    oh0[:], inpb_f[:], iota_f[:, :1], None, AOT.is_equal
                )
                nc.vector.tensor_scalar(
                    oh1[:], inpb_f[:], iota_f_hi[:, :1], None, AOT.is_equal
                )
                return oh0, oh1

            # ---- setup: input casts, table gathers, weights, one-hots
            inpr_b = [inpr_cast(0), inpr_cast(1)]
            t2_b = [gather_t2(0), gather_t2(1)]
            for c in range(EC):
                diag_chunk(c)
            oh_b = [onehot(0, inpr_b[0]), onehot(1, inpr_b[1])]

            ccat_b = []
            for b in range(BPC):
                t2 = t2_b[b]

                # ---- 11-tap conv on TensorE, output directly in [p, e]:
                # out[p', e'] = sum_i t2[i, c, off+p'+k] * diag_ck[i, e']
                ccat0 = mmp.tile([128, E], bf16, tag=f"c0_{b}")
                ccat1 = mmp.tile([128, E], bf16, tag=f"c1_{b}")
                nc.vector.memzero(ccat1[:])
                # groups: (pc, c-range, drain engine)
                groups = (
                    (0, range(0, 3), "v"),
                    (0, range(3, EC), "v"),
                    (1, range(0, 3), "v"),
                    (1, range(3, EC), "v"),
                )
                for pc, crange, eng in groups:
                    pcw = 128 if pc == 0 else P - 128
                    gw = len(crange) * 128
                    psc = psumc.tile([128, 512], f32, tag="pc")
                    for ci, c in enumerate(crange):
                        for k in range(W):
                            nc.tensor.matmul(
                                out=psc[:pcw, ci * 128 : (ci + 1) * 128],
                                lhsT=t2[:, c, k + pc * 128 : k + pc * 128 + pcw],
                                rhs=diagb[:, c * W + k, :],
                                start=(k == 0),
                                stop=(k == W - 1),
                            )
                    cdst = ccat0 if pc == 0 else ccat1
                    c0 = crange.start * 128
                    if eng == "v":
                        nc.vector.tensor_copy(
                            cdst[:pcw, c0 : c0 + gw], psc[:pcw, :gw]
                        )
                    else:
                        nc.scalar.copy(
                            cdst[:pcw, c0 : c0 + gw], psc[:pcw, :gw]
                        )
                ccat_b.append((ccat0, ccat1))
                if debug and b == 0:
                    nc.gpsimd.dma_start(out=dbg["ccat0"][:], in_=ccat0[:])

                oh0, oh1 = oh_b[b]
                # ---- gather matmul: out[s, e] = sum_p oh[p, s] * C[p, e]
                for m in range(MTILES):
                    mw = min(128, S - m * 128)
                    pso = psummm.tile([128, E], f32, tag="po")
                    for oh, cc, st in ((oh0, ccat0, True), (oh1, ccat1, False)):
                        for n0, nw in ((0, 512), (512, 256)):
                            nc.tensor.matmul(
                                out=pso[:mw, n0 : n0 + nw],
                                lhsT=oh[:, m * 128 : m * 128 + mw],
                                rhs=cc[:, n0 : n0 + nw],
                                start=st,
                                stop=not st,
                            )
                    ob = obp.tile([128, E], f32, tag="ob")
                    if m % 3 == 0:
                        nc.vector.tensor_copy(ob[:mw, :], pso[:mw, :])
                    else:
                        nc.scalar.copy(ob[:mw, :], pso[:mw, :])
                    nc.sync.dma_start(
                        out=out[b * S + m * 128 : b * S + m * 128 + mw, :],
                        in_=ob[:mw, :],
                    )

    nc.finalize()
    return nc


def _get_nc():
    if "nc" not in _cache:
        _cache["nc"] = _build()
    return _cache["nc"]


def _prep_shared(data, w):
    # layout-only host staging (no arithmetic)
    d0 = np.asarray(data, dtype=np.float32)[:, :, 0, :]  # [100, 166, 768]
    dT = np.transpose(d0, (0, 2, 1))  # [100, 768, 166]
    dT = (
        dT.reshape(NSNIP, EC, 128, P)
        .transpose(0, 2, 1, 3)
        .reshape(NSNIP * 128, EC * P)
    )
    dataT2 = np.ascontiguousarray(dT, dtype=np.float32)
    wT = np.asarray(w, dtype=np.float32).T  # [768, 11]
    w2 = wT.reshape(EC, 128, W).transpose(1, 0, 2)  # [128, EC, W]
    import ml_dtypes

    diagw = np.zeros((128, EC * W, 128), dtype=ml_dtypes.bfloat16)
    ii = np.arange(128)
    diagw[ii, :, ii] = w2.reshape(128, EC * W).astype(ml_dtypes.bfloat16)
    diagw = np.ascontiguousarray(diagw.reshape(128, EC * W * 128))
    return dataT2, diagw


def kernel(inputs, code_snippet_id, data, w, _trace=False):
    from concourse.bass_utils import run_bass_kernel_spmd

    nc = _get_nc()
    inputs = np.asarray(inputs, dtype=np.int32)
    code_snippet_id = np.asarray(code_snippet_id, dtype=np.int32)
    dataT2, diagw = _prep_shared(data, w)

    in_maps = []
    for ci in range(N_CORES):
        b0 = ci * BPC
        in_maps.append(
            {
                "meta": np.ascontiguousarray(
                    np.concatenate(
                        [
                            code_snippet_id[b0 : b0 + BPC].reshape(-1),
                            inputs[b0 : b0 + BPC].reshape(-1),
                        ]
                    ).reshape(1, -1)
                ),
                "dataT2": dataT2,
                "diagw": diagw,
            }
        )

    res = run_bass_kernel_spmd(
        nc, in_maps, core_ids=list(range(N_CORES)), trace=_trace
    )
    _cache["last_results"] = res
    out = np.concatenate(
        [res.results[i]["out"].reshape(BPC, S, E) for i in range(N_CORES)],
        axis=0,
    ).astype(np.float32)
    return out
